# revision 2
# baseline (speedup 1.0000x reference)
"""Causal self-attention on 8 Trainium2 NeuronCores — v2.

Sharding: core c handles batch b = c//2 and head-group g = c%2 (8 of 16
heads), as in v1. Changes vs v1:

- All weights/activations stream in bf16 (half the DMA bytes).
- Batched DMA loads (one DMA per tensor), spread across SP/Act/Pool
  engines so transfers overlap instead of serializing on SP.
- PV matmuls put q on PSUM partitions (out [128q, 65-moving] per head)
  which halves their PE cost vs the [65, q-moving] layout, and makes the
  softmax normalization a per-partition tensor_scalar (cheap) instead of
  a reciprocal-broadcast matmul chain.
- All eight PV/denominator accumulation chains of a (q-tile, head-pair)
  share one PSUM bank each (exact 2KB), exploiting the 2KB zero-region:
  only the first matmul into the bank uses start=True; the rest
  accumulate onto pending-zero bytes.
- y is transposed back to feature-major with XBAR DMA transposes (16x128
  tiles) instead of PE matmul transposes.
- c_proj is ROW-sharded: each core multiplies its own y half against its
  512 rows of w_proj, producing partial sums for all 1024 output columns
  (plus half the bias), and a per-q-tile pairwise ReduceScatter(add) over
  bf16 partials reduces them and writes each core's 512 output columns
  straight into the kernel output. This removes the y allgather + reload
  hop entirely: c_proj consumes y from SBUF right after each attention
  tile, and the exchange overhead drops (the collective moves only the
  peer-bound half, and nothing runs after it but the next tile).
- The softmax exp is split across Act (exact exp) and DVE/Pool (bf16
  Schraudolph exp) in the later, bigger attention tiles.
- QKV projection, attention and c_proj are software-pipelined: chains of
  the next token-chunk's projection are interleaved into the attention
  emission as PE fillers, so PE stays busy while the exp engines chew.
"""

import math

import numpy as np

B, T, C, H = 4, 2048, 1024, 16
D = C // H            # 64
NCORES = 8
GROUPS = [[0, 1], [2, 3], [4, 5], [6, 7]]
QT = 512              # q-tile width
KB = 128              # k-block size
NQT = T // QT         # 4
HPAIRS = 4            # head pairs per core

_CACHE = {}
HOST_BIAS = True


# --------------------------------------------------------------------------
# walrus workaround: this toolchain allows only ONE sync-wait per
# instruction. Split the end-of-kernel drain, and hoist excess waits from
# any instruction onto NoOps inserted just before it (same engine).
# --------------------------------------------------------------------------
def _patched_tc_class():
    import concourse.tile as tile
    from concourse.vector_clock import ScopedClock, VectorClock

    class PatchedTileContext(tile.TileContext):
        def _drain_and_barrier(self, tick_clock, wait_clock):
            gc = tick_clock.global_clock
            n = len(gc)
            ahead = [p for p in range(n) if gc[p] > 0]
            for p in ahead:
                vec = [gc[q] if q == p else 0 for q in range(n)]
                inst = self.nc.sync.drain()
                wait_clock.add_sem_waits(
                    inst.ins, ScopedClock({None: VectorClock(vec)})
                )
            if not ahead:
                inst = self.nc.sync.drain()
                wait_clock.add_sem_waits(
                    inst.ins, ScopedClock({None: tick_clock.global_clock})
                )
            self.nc.all_engine_barrier()
            assert self.sems is not None
            popped = self.nc._tile_sem_poison_stack.pop()
            assert popped is self._sem_poison
            self.nc.clear_and_free_semaphores(list(self.sems.allocated().values()))
            self.nc.all_engine_barrier()

    return PatchedTileContext


def _split_sync_waits(nc, max_waits=1):
    import concourse.mybir as mybir

    k = 0
    for f in nc.m.functions:
        for bb in f.blocks:
            newl = []
            dirty = False
            for inst in bb.instructions:
                si = inst.sync_info
                if si is not None and len(si.on_wait) > max_waits:
                    waits = list(si.on_wait)
                    excess, keep = waits[:-max_waits], waits[-max_waits:]
                    for w in excess:
                        k += 1
                        nop = mybir.InstNoOp(
                            name=f"I-waitsplit-{k}", ins=[], outs=[]
                        )
                        nop.engine = inst.engine
                        nop.sync_info = mybir.SyncInfo(on_wait=[w], on_update=[])
                        newl.append(nop)
                    inst.sync_info = mybir.SyncInfo(
                        on_wait=keep, on_update=si.on_update
                    )
                    dirty = True
                newl.append(inst)
            if dirty:
                bb.instructions = newl
    return k


# --------------------------------------------------------------------------
# the Bass program (identical on all 8 cores; only input data differs)
# --------------------------------------------------------------------------
def _build_nc(split_waits=True):
    import concourse.bass as bass
    import concourse.mybir as mybir

    F32 = mybir.dt.float32
    F32R = mybir.dt.float32r
    BF16 = mybir.dt.bfloat16
    I16 = mybir.dt.int16
    EXP = mybir.ActivationFunctionType.Exp
    COPY = mybir.ActivationFunctionType.Copy
    MULT = mybir.AluOpType.mult
    ADD = mybir.AluOpType.add

    # bf16 Schraudolph exp: exp(x) ~= bitcast_bf16(int16(x*A + B)); the
    # int16 convert truncates, B was tuned for that. Max rel err ~3.3%,
    # which the softmax normalization mostly cancels. Lets DVE/Pool share
    # the softmax exp work with the Act engine.
    SCH_A = 128.0 / math.log(2.0)
    SCH_B = 16251.0

    PatchedTileContext = _patched_tc_class()

    nc = bass.Bass()

    # ---- parameters --------------------------------------------------
    xT_p = nc.declare_dram_parameter("xT", [C, T], BF16, isOutput=False)
    wqk_p = nc.declare_dram_parameter("wqk", [C, 1024], BF16, isOutput=False)
    wv_p = nc.declare_dram_parameter("wv", [C, 512], BF16, isOutput=False)
    # w_proj ROWS for this core's 512 y-features, all 1024 output columns
    wp_p = nc.declare_dram_parameter("wp", [512, 1024], BF16, isOutput=False)
    bqk_p = nc.declare_dram_parameter("bqk", [128, 8], F32, isOutput=False)
    bv_p = nc.declare_dram_parameter("bv", [1, 512], F32R, isOutput=False)
    mask_p = nc.declare_dram_parameter("masks", [128, 128], BF16, isOutput=False)
    ident_p = nc.declare_dram_parameter("ident", [128, 128], BF16, isOutput=False)
    # bf16 output: the ReduceScatter writes it directly; host upcasts
    out_p = nc.declare_dram_parameter("out", [T, 512], BF16, isOutput=True)

    with PatchedTileContext(nc) as tc:
        dram_cm = tc.tile_pool(name="dramp", bufs=1, space="DRAM")
        dram = dram_cm.__enter__()
        # per-q-tile c_proj partial sums, chunk-major for the pairwise
        # ReduceScatter: prs[g, t, c] = partial[t, g*512 + c]
        prs = [
            dram.tile([2, QT, 512], BF16, name=f"prs{qt}", tag=f"prs{qt}")
            for qt in range(NQT)
        ]
        # ReduceScatter can't write IO tensors on HW; bounce via DRAM
        rsout = [
            dram.tile([QT, 512], BF16, name=f"rsout{qt}", tag=f"rsout{qt}")
            for qt in range(NQT)
        ]

        persist_cm = tc.tile_pool(name="persist", bufs=1)
        persist = persist_cm.__enter__()

        # ---- loads: xt0 on Act first, weights on SP, wp on Pool ------
        xt_sb = [
            persist.tile([128, 8, 1024], BF16, name=f"xt{th}", tag=f"xt{th}")
            for th in range(2)
        ]
        # halves split across Act/SP so the first qk chain's inputs land
        # in ~3us instead of ~6
        nc.scalar.dma_start(
            xt_sb[0][:, 0:4],
            xT_p[0:512, 0:1024].rearrange("(kc p) t -> p kc t", p=128),
        )
        wqk_sb = persist.tile([128, 8, 1024], BF16)
        nc.sync.dma_start(
            wqk_sb[:, 0:4],
            wqk_p[0:512, :].rearrange("(kc p) f -> p kc f", p=128),
        )
        nc.scalar.dma_start(
            xt_sb[0][:, 4:8],
            xT_p[512:1024, 0:1024].rearrange("(kc p) t -> p kc t", p=128),
        )
        nc.sync.dma_start(
            wqk_sb[:, 4:8],
            wqk_p[512:1024, :].rearrange("(kc p) f -> p kc f", p=128),
        )
        bqk_sb = persist.tile([128, 8], F32)
        nc.scalar.dma_start(bqk_sb[:], bqk_p[:])
        bv_sb = persist.tile([1, 512], F32R)
        nc.scalar.dma_start(bv_sb[:], bv_p[:])
        mask_sb = persist.tile([128, 128], BF16)
        nc.scalar.dma_start(mask_sb[:], mask_p[:])
        wv_sb = persist.tile([128, 8, 512], BF16)
        nc.sync.dma_start(
            wv_sb[:], wv_p[:].rearrange("(kc p) f -> p kc f", p=128)
        )
        ident = persist.tile([128, 128], BF16)
        nc.scalar.dma_start(ident[:], ident_p[:])
        wp_sb = persist.tile([128, 4, 1024], BF16)
        nc.gpsimd.dma_start(
            wp_sb[:], wp_p[:].rearrange("(kc p) f -> p kc f", p=128)
        )

        ones_row = persist.tile([1, 128], F32R)
        nc.vector.memset(ones_row[:].bitcast(F32), 1.0)
        ones_col = persist.tile([128, 1], BF16)
        nc.vector.memset(ones_col[:], 1.0)
        bv_b = persist.tile([128, 512], F32R)

        # ---- persistent activations ----------------------------------
        # qk_sb[ft]: feature-tile ft of [Q^T | K^T], [128, T] bf16;
        # ft 0..3 = Q (head pair ft), ft 4..7 = K.
        qk_sb = [
            persist.tile([128, T], BF16, name=f"qk{ft}", tag=f"qk{ft}")
            for ft in range(8)
        ]
        # v_sb[tt]: [128, 8, 65] bf16 — T-chunk tt of V per local head + ones
        v_sb = [
            persist.tile([128, 8, 65], BF16, name=f"v{tt}", tag=f"v{tt}")
            for tt in range(16)
        ]
        for tt in range(16):
            nc.vector.memset(v_sb[tt][:, :, 64], 1.0)

        # ---- pools ---------------------------------------------------
        mm_cm = tc.tile_pool(name="mm", bufs=1, space="PSUM")
        mm = mm_cm.__enter__()
        att_ps_cm = tc.tile_pool(name="attps", bufs=1, space="PSUM")
        att_ps = att_ps_cm.__enter__()
        attn_cm = tc.tile_pool(name="attn", bufs=1)
        attn = attn_cm.__enter__()
        cp_cm = tc.tile_pool(name="cp", bufs=1)
        cp = cp_cm.__enter__()

        # ---- emitters ------------------------------------------------
        def emit_qk_chain(c, ft):
            th, half = divmod(c, 2)
            ps = mm.tile([128, 512], F32, tag="mm", bufs=2)
            for kc in range(8):
                nc.tensor.matmul(
                    ps[:],
                    wqk_sb[:, kc, ft * 128 : (ft + 1) * 128],
                    xt_sb[th][:, kc, half * 512 : (half + 1) * 512],
                    start=(kc == 0),
                    stop=(kc == 7),
                )
            nc.vector.tensor_scalar_add(
                out=qk_sb[ft][:, c * 512 : (c + 1) * 512],
                in0=ps[:],
                scalar1=bqk_sb[:, ft : ft + 1],
            )

        def emit_v_chain(c, i):
            th, half = divmod(c, 2)
            tt16 = 4 * c + i
            ps = mm.tile([128, 512], F32, tag="mm", bufs=2)
            for kc in range(8):
                nc.tensor.matmul(
                    ps[:],
                    xt_sb[th][:, kc, half * 512 + i * 128 : half * 512 + (i + 1) * 128],
                    wv_sb[:, kc, :],
                    start=(kc == 0),
                    stop=(kc == 7),
                )
            nc.vector.tensor_tensor(
                out=v_sb[tt16][:, :, 0:64],
                in0=ps[:].rearrange("p (h d) -> p h d", h=8),
                in1=bv_b[:].rearrange("p (h d) -> p h d", h=8),
                op=ADD,
            )

        yT = {}

        def emit_cproj_chain(qt, tnl, g):
            # partial c_proj for token tile tnl of q-tile qt, output column
            # half g, contracting this core's own 512 y-features (from the
            # yT SBUF tiles of att(qt) — no DRAM roundtrip)
            ps = mm.tile([128, 512], F32, tag="mm", bufs=2)
            for hp in range(4):
                nc.tensor.matmul(
                    ps[:],
                    yT[(qt, hp)][:, tnl * 128 : (tnl + 1) * 128],
                    wp_sb[:, hp, g * 512 : (g + 1) * 512],
                    start=(hp == 0),
                    stop=(hp == 3),
                )
            # convert psum->bf16 on Act (gpsimd can't read PSUM on HW);
            # the c_proj bias is added on the host instead
            ot = cp.tile([128, 512], BF16, tag="ot", bufs=2)
            nc.scalar.activation(ot[:], ps[:], COPY)
            nc.sync.dma_start(
                prs[qt][g, tnl * 128 : (tnl + 1) * 128, :], ot[:]
            )

        def emit_rs(qt, half):
            # pairwise ReduceScatter over one token-half of the tile's
            # partials: finer pieces let the first start while the second
            # half's c_proj chains still run. Pool carries only
            # collectives (+ the wp load), so parking it costs nothing.
            if half == 1:
                return
            nc.gpsimd.collective_compute(
                "ReduceScatter",
                ADD,
                replica_groups=GROUPS,
                ins=[prs[qt][:].opt()],
                outs=[rsout[qt][:].opt()],
            )

        fillers = []

        def drain_fillers(n):
            for _ in range(n):
                if not fillers:
                    return
                fillers.pop(0)()

        # ---- phase 0: first token-chunk QKV --------------------------
        for ft in range(8):
            emit_qk_chain(0, ft)

        # bias broadcasts (K=1 matmuls) after the qk chains so PE starts
        # on real work first
        bcv = mm.tile([128, 512], F32, tag="mm", bufs=2)
        nc.tensor.matmul(bcv[:], ones_row[:], bv_sb[:], start=True, stop=True)
        nc.scalar.activation(bv_b[:], bcv[:], COPY)

        for i in range(4):
            emit_v_chain(0, i)

        # xt th=1 prefetch (SP) for chunks 2,3
        nc.sync.dma_start(
            xt_sb[1][:], xT_p[:, 1024:2048].rearrange("(kc p) t -> p kc t", p=128)
        )

        # ---- phases: attention + pipelined fillers -------------------
        for qt in range(NQT):
            q0 = qt * QT
            nkb = 4 * qt + 4
            # fill the queue for this attention tile
            if qt < 3:
                c = qt + 1
                for ft in range(8):
                    fillers.append(lambda c=c, ft=ft: emit_qk_chain(c, ft))
                for i in range(4):
                    fillers.append(lambda c=c, i=i: emit_v_chain(c, i))
            blk = 0
            for hp in range(HPAIRS):
                accy = att_ps.tile([128, 8, 64], F32, tag="accy", bufs=1)
                accd = att_ps.tile([128, 8], F32, tag="accd", bufs=1)
                state = {"first_y": True, "first_d": True}

                def pv_block(kb, m, p2, hp=hp, qt=qt, accy=accy, accd=accd,
                             state=state):
                    for j in range(max(m, 0), 4):
                        jq = j * 128
                        for h in (0, 1):
                            col = 2 * j + h
                            nc.tensor.matmul(
                                accy[:, col, :],
                                p2[:, h, jq : jq + 128],
                                v_sb[kb][:, 2 * hp + h, 0:64],
                                start=state["first_y"],
                                stop=(kb == 4 * qt + j),
                                skip_group_check=True,
                            )
                            state["first_y"] = False
                            nc.tensor.matmul(
                                accd[:, col : col + 1],
                                p2[:, h, jq : jq + 128],
                                ones_col[:],
                                start=state["first_d"],
                                stop=(kb == 4 * qt + j),
                                skip_group_check=True,
                            )
                            state["first_d"] = False

                # software-pipelined: emit S(kb) before PV(kb-1) so PE
                # isn't parked on exp(kb-1) before it can start S(kb) —
                # with PV first the whole S->exp->PV chain serializes
                prev = None
                for kb in range(nkb):
                    m = kb - 4 * qt
                    off = 0 if m < 0 else KB * m
                    s2 = att_ps.tile([128, 2, QT], F32, tag="s2", bufs=2)
                    for h in (0, 1):
                        nc.tensor.matmul(
                            s2[:, h, off:QT],
                            qk_sb[4 + hp][h * 64 : (h + 1) * 64, kb * KB : (kb + 1) * KB],
                            qk_sb[hp][h * 64 : (h + 1) * 64, q0 + off : q0 + QT],
                            start=True,
                            stop=True,
                        )
                    if prev is not None:
                        pv_block(*prev)
                    p2 = attn.tile([128, 2, QT], BF16, tag="p2", bufs=3)
                    # route exp: Act by default; DVE takes a Schraudolph
                    # share in the bigger tiles so neither engine trails
                    if qt in (1, 2):
                        eng = ("act", "act", "dve")[blk % 3]
                    elif qt == 3:
                        eng = ("act", "dve")[blk % 2]
                    else:
                        eng = "act"
                    blk += 1
                    if eng == "act":
                        nc.scalar.activation(
                            p2[:, :, off:QT], s2[:, :, off:QT], EXP
                        )
                    else:
                        nc.vector.tensor_scalar(
                            out=p2[:, :, off:QT].bitcast(I16),
                            in0=s2[:, :, off:QT],
                            scalar1=SCH_A,
                            scalar2=SCH_B,
                            op0=MULT,
                            op1=ADD,
                        )
                    if m >= 0:  # triangle mask on the diagonal strip
                        nc.vector.tensor_tensor(
                            out=p2[:, :, off : off + 128],
                            in0=p2[:, :, off : off + 128],
                            in1=mask_sb[:].unsqueeze(1).broadcast_to([128, 2, 128]),
                            op=MULT,
                        )
                    prev = (kb, m, p2)
                    if kb % 3 == 2:
                        drain_fillers(1)
                pv_block(*prev)
                # ---- normalize + transpose + store -------------------
                r8 = attn.tile([128, 8], F32, tag="r8", bufs=2)
                with nc.allow_low_precision(reason="softmax recip"):
                    nc.vector.reciprocal(r8[:], accd[:])
                # bufs=4 (one slot per q-tile): reusing a slot would make
                # these transposes wait on the previous user's ldweights,
                # which have no real semaphores — the framework then waits
                # on the whole downstream chain incl. a ReduceScatter
                ytile = attn.tile(
                    [128, QT], BF16, name=f"yT{qt}_{hp}", tag=f"yT{hp}", bufs=4
                )
                yT[(qt, hp)] = ytile
                yqall = attn.tile([128, 8, 64], BF16, tag="yqall", bufs=2)
                nc.vector.tensor_tensor(
                    out=yqall[:],
                    in0=accy[:],
                    in1=r8[:].unsqueeze(2).broadcast_to([128, 8, 64]),
                    op=MULT,
                )
                # transpose y back to feature-major on PE (identity-rhs
                # transpose matmuls into one psum bank, bf16 quadrants),
                # then one Pool copy to SBUF. XBAR DMA transposes would be
                # cheaper but the scheduler serializes them against the
                # collectives, which wrecks the pipeline.
                tpp = mm.tile([128, 512], F32, tag="mm", bufs=2)
                for j in range(4):
                    nc.tensor.matmul(
                        tpp[:, j * 64 : (j + 1) * 64].bitcast(BF16),
                        yqall[:, 2 * j : 2 * j + 2, :],
                        ident[:],
                        start=(j == 0),
                        stop=(j == 3),
                        is_transpose=True,
                        skip_group_check=True,
                    )
                nc.vector.tensor_copy(ytile[:], tpp[:, 0:256].bitcast(BF16))
                drain_fillers(1)
            drain_fillers(len(fillers))
            # deferred by one tile: emitting a ReduceScatter before the
            # next attention tile makes its DMA-semaphore updates precede
            # that tile's transposes in the counter numbering, so readers
            # of the transposes would transitively wait on the collective
            if qt > 0:
                emit_rs(qt - 1, 0)
            if qt > 1:
                # copy the tile reduced two iterations ago out to the IO
                # tensor now — only the last tile's copy stays on the tail
                nc.gpsimd.dma_start(
                    out_p[(qt - 2) * QT : (qt - 1) * QT, :], rsout[qt - 2][:]
                )
            # ---- c_proj partials for this q-tile (y is in SBUF) ------
            for tnl in range(4):
                for g in (0, 1):
                    emit_cproj_chain(qt, tnl, g)
        emit_rs(3, 0)
        for qt in (2, 3):
            nc.gpsimd.dma_start(
                out_p[qt * QT : (qt + 1) * QT, :], rsout[qt][:]
            )

        cp_cm.__exit__(None, None, None)
        attn_cm.__exit__(None, None, None)
        att_ps_cm.__exit__(None, None, None)
        mm_cm.__exit__(None, None, None)
        persist_cm.__exit__(None, None, None)
        dram_cm.__exit__(None, None, None)

    if split_waits:
        _split_sync_waits(nc)
    return nc


# --------------------------------------------------------------------------
# host side
# --------------------------------------------------------------------------
def _make_masks():
    import ml_dtypes

    i = np.arange(128)[:, None]
    j = np.arange(128)[None, :]
    return (i <= j).astype(ml_dtypes.bfloat16)  # [128, 128] triangle


def _prep_core_inputs(x, w_attn, b_attn, w_proj, b_proj):
    import ml_dtypes

    Bb = ml_dtypes.bfloat16
    masks = _make_masks()
    in_maps = []
    for c in range(NCORES):
        b, g = divmod(c, 2)
        sl = slice(512 * g, 512 * (g + 1))
        wq = w_attn[:, 0 * C :][:, sl] * 0.125  # fold 1/sqrt(D)
        wk = w_attn[:, C : 2 * C][:, sl]
        bq = b_attn[0 * C :][sl] * 0.125
        bk = b_attn[C : 2 * C][sl]
        wqk = np.concatenate([wq, wk], axis=1)          # [C, 1024]
        bqk = np.concatenate([bq, bk]).reshape(8, 128).T  # [128, 8]
        in_maps.append(
            {
                "xT": np.ascontiguousarray(x[b].T).astype(Bb),
                "wqk": np.ascontiguousarray(wqk).astype(Bb),
                "wv": np.ascontiguousarray(w_attn[:, 2 * C :][:, sl]).astype(Bb),
                # w_proj ROWS for this core's y-features, all output cols
                "wp": np.ascontiguousarray(w_proj[sl, :]).astype(Bb),
                "bqk": np.ascontiguousarray(bqk).astype(np.float32),
                "bv": b_attn[2 * C :][sl].reshape(1, 512).astype(np.float32),
                "masks": masks,
                "ident": np.eye(128, dtype=np.float32).astype(Bb),
            }
        )
    return in_maps


def _make_compiled(nc):
    """Build a reusable jitted SPMD callable (mirrors
    bass2jax.run_bass_via_pjrt's multi-core branch, but cached so repeat
    calls don't re-trace)."""
    import jax
    import concourse.mybir as mybir
    from jax.experimental.shard_map import shard_map
    from jax.sharding import Mesh, PartitionSpec
    from concourse import bass2jax

    bass2jax.install_neuronx_cc_hook()
    partition_name = (
        nc.partition_id_tensor.name if nc.partition_id_tensor else None
    )
    in_names, out_names, out_avals, zero_shapes = [], [], [], []
    for alloc in nc.m.functions[0].allocations:
        if not isinstance(alloc, mybir.MemoryLocationSet):
            continue
        name = alloc.memorylocations[0].name
        if alloc.kind == "ExternalInput":
            if name != partition_name:
                in_names.append(name)
        elif alloc.kind == "ExternalOutput":
            out_names.append(name)
            shape = tuple(alloc.tensor_shape)
            dtype = mybir.dt.np(alloc.dtype)
            out_avals.append(jax.core.ShapedArray(shape, dtype))
            zero_shapes.append((shape, dtype))
    n_params = len(in_names)
    in_names_full = list(in_names) + list(out_names)
    if partition_name is not None:
        in_names_full.append(partition_name)
    donate = tuple(range(n_params, n_params + len(out_names)))

    def _body(*args):
        operands = list(args)
        if partition_name is not None:
            operands.append(bass2jax.partition_id_tensor())
        outs = bass2jax._bass_exec_p.bind(
            *operands,
            out_avals=tuple(out_avals),
            in_names=tuple(in_names_full),
            out_names=tuple(out_names),
            lowering_input_output_aliases=(),
            sim_require_finite=True,
            sim_require_nnan=True,
            nc=nc,
        )
        return tuple(outs)

    devices = jax.devices()[:NCORES]
    mesh = Mesh(np.asarray(devices), ("core",))
    in_specs = (PartitionSpec("core"),) * (n_params + len(out_names))
    out_specs = (PartitionSpec("core"),) * len(out_names)
    sharded = jax.jit(
        shard_map(
            _body, mesh=mesh, in_specs=in_specs, out_specs=out_specs,
            check_rep=False,
        ),
        donate_argnums=donate,
        keep_unused=True,
    )
    return {
        "sharded": sharded,
        "in_names": in_names,
        "out_names": out_names,
        "out_avals": out_avals,
        "zero_shapes": zero_shapes,
        "mesh": mesh,
    }


def _get_compiled():
    if "compiled" not in _CACHE:
        _CACHE["compiled"] = _make_compiled(_build_nc())
    return _CACHE["compiled"]


def _concat_inputs(cc, in_maps):
    arrs = []
    for name in cc["in_names"]:
        arrs.append(
            np.concatenate([np.asarray(m[name]) for m in in_maps], axis=0)
        )
    return arrs


def _zeros(cc):
    return [
        np.zeros((NCORES * shape[0], *shape[1:]), dtype)
        for shape, dtype in cc["zero_shapes"]
    ]


def run_spmd(in_maps):
    """Returns an object with .results: list of per-core {name: array}."""
    cc = _get_compiled()
    out_arrs = cc["sharded"](*_concat_inputs(cc, in_maps), *_zeros(cc))
    results = []
    for c in range(NCORES):
        d = {}
        for i, name in enumerate(cc["out_names"]):
            shape = cc["out_avals"][i].shape
            d[name] = np.asarray(out_arrs[i]).reshape(NCORES, *shape)[c]
        results.append(d)

    class _R:
        pass

    r = _R()
    r.results = results
    return r


def kernel(x, w_attn, b_attn, w_proj, b_proj):
    x = np.asarray(x, dtype=np.float32)
    w_attn = np.asarray(w_attn, dtype=np.float32)
    b_attn = np.asarray(b_attn, dtype=np.float32)
    w_proj = np.asarray(w_proj, dtype=np.float32)
    b_proj = np.asarray(b_proj, dtype=np.float32)

    in_maps = _prep_core_inputs(x, w_attn, b_attn, w_proj, b_proj)
    res = run_spmd(in_maps)
    out = np.empty((B, T, C), dtype=np.float32)
    for b in range(B):
        out[b, :, 0:512] = res.results[2 * b]["out"]
        out[b, :, 512:1024] = res.results[2 * b + 1]["out"]
    out += b_proj  # c_proj bias applied on host
    return out
